# revision 18
# baseline (speedup 1.0000x reference)
"""Trainium2 Bass kernel for nn_AttentionHead_6786048328376.

8-head spatial attention block: q/k/v 1x1-conv projections with additive
positional embedding on q/k, softmax over the QUERY axis (dim=2), attention
apply, channel-major output, 2-layer 1x1-conv MLP with mish, residual add.

Sharding: pure data-parallel over batch - 8 batch elements, one per
NeuronCore. Weights are replicated; no collectives.

Per-core design (C=512, N=H*W=1024, 8 heads, dh=32, ch=64):
  - The scalar (ACT) engine is the fundamental bottleneck: 8M exp elements
    per core = ~71us of ACTIVATE time at 1 el/cycle/lane.  Everything else
    is scheduled to hide underneath it.
  - x is held channel-major [512, 1024]; q/k land head-stacked [256, 1024]
    (row = 32*head + d) so head-pairs sit on 32-row PE array strips ->
    scores use 2-way row-tiled K=32 matmuls (tile_position).
  - scores are computed TRANSPOSED: sT[m, n] (key-major) so the softmax
    reduction over the query axis n is a free-axis reduction. Both n-halves
    of one head land in one 2-bank psum tile, so exp is a single [128,1024]
    scalar-engine pass per (head, m-chunk) writing bf16 (no max subtraction
    needed: |scores| <= ~8 at this problem's scale).
  - row-sums of exp are split between the scalar engine (fused accum_out)
    and the vector engine (tensor_reduce over the bf16 exp tile) to balance
    the two engines.
  - v is computed directly transposed vT[n, c] = x.T @ Wv.T, just-in-time
    inside pair-group 0; the softmax 1/sum is folded into vT rows (64
    els/row) instead of dividing the 1M-el score matrix.
  - attention apply uses 2-way col-tiled bf16 matmuls (both heads of the
    pair concurrent, M=64) accumulating over m-chunks, lagged one m-chunk
    behind the exps for pipeline smoothness.
  - prologue: DMA is per-queue descriptor-rate limited (~1 descriptor per
    partition line; wide lines amortize much better) and partition-sliced
    writes are pathologically slow, so the weights are HOST-PACKED into a
    few wide [128, X] tensors, one full-width DMA each, spread over the
    sync/scalar/gpsimd queues in gating order: xbf | wqk(t0,t2)+pe(t0,t2) |
    wv+wqk(t1,t3)+pe(t1,t3) | w1+w2.  bv arrives as a [1,512] row (1
    descriptor) and is broadcast on-chip with a ones-column matmul.
  - a burst of dummy matmuls on a zeroed tile warms the PE HAM clock gate
    during the DMA wait so the projections run at full clock.
  - q/k proj of tiles 1,3 is slotted inside pair-group 0 so only tiles 0,2
    gate the first exp.
  - residual add reads the bf16 xbf copy (no separate fp32 x load).
  - mish(x) = x*tanh(ln(1+exp(x))) via Exp -> Ln(bias=1) -> Tanh on the
    scalar engine (phased to avoid activation-table thrash) plus vector ops.
"""

import numpy as np

_CACHE = {}

# of the 8 m-chunks per (head, pair-group), how many use ACT accum_out for
# the exp row-sum; the rest use a DVE tensor_reduce over the bf16 exp tile.
ACT_ACCUM_PER_8 = 4
WARMUP_MMS = 22
AV_LAG = 1


def _build():
    import concourse.bacc as bacc
    import concourse.tile as tile
    import concourse.mybir as mybir

    dt = mybir.dt
    F32 = dt.float32
    BF16 = dt.bfloat16
    Act = mybir.ActivationFunctionType
    Alu = mybir.AluOpType
    AxX = mybir.AxisListType.X

    nc = bacc.Bacc("TRN2", target_bir_lowering=False, debug=False)

    # host-packed wide tensors (see _make_in_maps for the layouts)
    xbf_d = nc.dram_tensor("xbf", [128, 4096], BF16, kind="ExternalInput").ap()
    pka_d = nc.dram_tensor("pka", [128, 3072], BF16, kind="ExternalInput").ap()
    pkb_d = nc.dram_tensor("pkb", [128, 9216], BF16, kind="ExternalInput").ap()
    bvr_d = nc.dram_tensor("bvr", [1, 512], BF16, kind="ExternalInput").ap()
    b1_d = nc.dram_tensor("b1", [128, 4], F32, kind="ExternalInput").ap()
    b2_d = nc.dram_tensor("b2", [128, 4], F32, kind="ExternalInput").ap()
    out_d = nc.dram_tensor("out", [512, 1024], F32, kind="ExternalOutput").ap()

    with tile.TileContext(nc) as tc:
        with tc.tile_pool(name="persist", bufs=1) as per, \
             tc.tile_pool(name="mtmp", bufs=18) as mt, \
             tc.tile_pool(name="etp", bufs=16) as etp, \
             tc.tile_pool(name="small", bufs=20) as sm, \
             tc.tile_pool(name="sbig", bufs=3, space="PSUM") as ps, \
             tc.tile_pool(name="av", bufs=2, space="PSUM") as av:

            def ptile(shape, dtype, name):
                return per.tile(shape, dtype, name=name, tag=name)

            xbf_sb = ptile([128, 4096], BF16, "xbfs")
            pka_sb = ptile([128, 3072], BF16, "pkas")
            pkb_sb = ptile([128, 9216], BF16, "pkbs")
            bvr_sb = ptile([1, 512], BF16, "bvrsb")
            bv_sb = ptile([128, 512], F32, "bvsb")
            b1_sb = ptile([128, 4], F32, "b1c")
            b2_sb = ptile([128, 4], F32, "b2c")
            qk_sb = [ptile([128, 1024], BF16, f"qks{i}") for i in range(4)]
            vt_sb = [ptile([128, 512], F32, f"vts{i}") for i in range(8)]
            attn_sb = [ptile([128, 1024], BF16, f"attn{i}") for i in range(4)]
            mish_sb = [ptile([128, 1024], BF16, f"mish{i}") for i in range(4)]
            out_sb = [ptile([128, 1024], F32, f"osb{i}") for i in range(4)]
            zr_sb = ptile([128, 512], BF16, "zrsb")
            one_sb = ptile([1, 128], BF16, "onesb")

            # wqk/pe blocks are laid out in gating order t = 0, 2, 1, 3;
            # t0/t2 live in pack A, t1/t3 (plus wv) in pack B
            ORD = {0: 0, 2: 1, 1: 2, 3: 3}

            def xbf(kc, c0, c1):
                return xbf_sb[:, 1024 * kc + c0:1024 * kc + c1]

            def wqk(t, kc):
                T = ORD[t]
                if T < 2:
                    c = 512 * T + 128 * kc
                    return pka_sb[:, c:c + 128]
                c = 2048 + 512 * (T - 2) + 128 * kc
                return pkb_sb[:, c:c + 128]

            def pe_t(t):
                T = ORD[t]
                if T < 2:
                    c = 1024 + 1024 * T
                    return pka_sb[:, c:c + 1024]
                c = 3072 + 1024 * (T - 2)
                return pkb_sb[:, c:c + 1024]

            def wv(kc):
                return pkb_sb[:, 512 * kc:512 * (kc + 1)]

            def w1(kc, c0, c1):
                return pkb_sb[:, 5120 + 512 * kc + c0:5120 + 512 * kc + c1]

            def w2(kc, c0, c1):
                return pkb_sb[:, 7168 + 512 * kc + c0:7168 + 512 * kc + c1]

            nc.sync.dma_start(out=xbf_sb, in_=xbf_d)
            nc.scalar.dma_start(out=pka_sb, in_=pka_d)
            nc.gpsimd.dma_start(out=pkb_sb, in_=pkb_d)
            nc.sync.dma_start(out=bvr_sb, in_=bvr_d)
            nc.sync.dma_start(out=b1_sb, in_=b1_d)
            nc.sync.dma_start(out=b2_sb, in_=b2_d)

            mm = nc.tensor.matmul

            # dummy matmuls on a zeroed tile warm the PE clock gate while the
            # gating DMAs stream in; a tiny anchor copy keeps the tile live.
            nc.vector.memset(zr_sb, 0.0)
            nc.vector.memset(one_sb, 1.0)
            wt = ps.tile([128, 512], F32, name="wps", tag="sbig")
            for _ in range(WARMUP_MMS):
                mm(wt, lhsT=zr_sb[:, 0:128], rhs=zr_sb, start=True, stop=True)
            wanchor = sm.tile([128, 1], F32, name="wanchor", tag="wanchor")
            nc.vector.tensor_copy(out=wanchor, in_=wt[:, 0:1])

            # q/k projections: qk[512, 1024] = WqkT.T @ x, then + (PE,
            # bias); the adds run per n-half so they overlap the next chain
            def proj_qk(t):
                pt = ps.tile([128, 1024], F32, name="pps", tag="sbig")
                for nh in range(2):
                    for kc in range(4):
                        mm(pt[:, 512 * nh:512 * (nh + 1)],
                           lhsT=wqk(t, kc),
                           rhs=xbf(kc, 512 * nh, 512 * (nh + 1)),
                           start=(kc == 0), stop=(kc == 3))
                    nc.vector.tensor_add(
                        qk_sb[t][:, 512 * nh:512 * (nh + 1)],
                        pt[:, 512 * nh:512 * (nh + 1)],
                        pe_t(t)[:, 512 * nh:512 * (nh + 1)])
            proj_qk(0)
            proj_qk(2)

            # bv broadcast [1,512] -> [128,512] via a ones-column matmul
            # (saves a descriptor-rate-limited 0.25MB DMA); emitted after
            # the gating projections so it never delays them in the PE FIFO
            bvp = ps.tile([128, 512], F32, name="bvp", tag="sbig")
            mm(bvp, lhsT=one_sb, rhs=bvr_sb, start=True, stop=True)
            nc.vector.tensor_copy(out=bv_sb, in_=bvp)

            def project_vt(i):
                # vT[n, c] = x.T @ WvT, then + bv - emitted just-in-time
                # inside the first pair-group so exp work starts early
                pt = ps.tile([128, 512], F32, name="vps", tag="sbig")
                for kc in range(4):
                    mm(pt, lhsT=xbf(kc, 128 * i, 128 * (i + 1)),
                       rhs=wv(kc),
                       start=(kc == 0), stop=(kc == 3))
                nc.vector.tensor_add(vt_sb[i], pt, bv_sb)

            # attention: four head-pair groups; scores + exp + row-sums with
            # the AV accumulation interleaved, lagged AV_LAG m-chunks behind
            # the exps for pipeline smoothness.
            for pg in range(4):
                g = pg // 2           # which 128-row q/k tile
                off0 = 64 * (pg % 2)  # partition offset of this pair in it
                q_t = qk_sb[g]
                k_t = qk_sb[2 + g]
                avt = [av.tile([128, 512], F32, name="avt", tag="av")
                       for _ in range(2)]  # [nh]
                S_all, ets = {}, {}

                def finish(mc):
                    # 1/rowsum, fold into vT slices, AV accumulation
                    R = sm.tile([128, 2], F32, name="R", tag="R")
                    nc.vector.reciprocal(R, S_all[mc])
                    for hp in range(2):
                        h = 2 * pg + hp
                        et = ets.pop((mc, hp))
                        vts = sm.tile([128, 64], BF16, name="vtsc", tag="vtsc")
                        nc.vector.tensor_scalar_mul(
                            vts, vt_sb[mc][:, 64 * h:64 * (h + 1)],
                            R[:, hp:hp + 1])
                        for nh in range(2):
                            # two col-tiled accumulation series share each
                            # bank on disjoint partition halves; has_written
                            # is per-element so this is safe - the sim's
                            # coarse zero-region tracker is what we skip.
                            mm(avt[nh][64 * hp:64 * hp + 64, :],
                               lhsT=vts,
                               rhs=et[:, 512 * nh:512 * (nh + 1)],
                               start=(mc == 0), stop=(mc == 7),
                               tile_position=(0, 64 * hp),
                               skip_group_check=True)

                for mc in range(8):
                    if pg == 0:
                        project_vt(mc)
                    if pg == 1 and mc == 1:
                        proj_qk(1)
                    if pg == 1 and mc == 4:
                        proj_qk(3)
                    S = sm.tile([128, 2], F32, name="S", tag="S")
                    S_all[mc] = S
                    for hp in range(2):
                        off = off0 + 32 * hp
                        sp = ps.tile([128, 1024], F32, name="sps", tag="sbig")
                        for nh in range(2):
                            mm(sp[:, 512 * nh:512 * (nh + 1)],
                               lhsT=k_t[off:off + 32, 128 * mc:128 * (mc + 1)],
                               rhs=q_t[off:off + 32, 512 * nh:512 * (nh + 1)],
                               start=True, stop=True,
                               tile_position=(off, 0))
                        et = etp.tile([128, 1024], BF16, name="et", tag="et")
                        if mc in (0, 1, 6, 7):
                            nc.scalar.activation(et, sp, Act.Exp,
                                                 accum_out=S[:, hp:hp + 1])
                        else:
                            nc.scalar.activation(et, sp, Act.Exp)
                            nc.vector.tensor_reduce(
                                S[:, hp:hp + 1], et, axis=AxX, op=Alu.add)
                        ets[(mc, hp)] = et
                    if mc >= AV_LAG:
                        finish(mc - AV_LAG)
                for mc in range(8 - AV_LAG, 8):
                    finish(mc)
                for nh in range(2):
                    nc.vector.tensor_copy(
                        out=attn_sb[pg][:, 512 * nh:512 * (nh + 1)],
                        in_=avt[nh])

            # MLP: h1 = W1 @ attn + b1; mish; out = W2 @ mish + b2 + x
            # full per-nh chains: MLP2 of nh=0 overlaps nh=1's mish chain
            h1f, t_t, sp_t, th_t = {}, {}, {}, {}
            for nh in range(2):
                for i in range(4):
                    pt = ps.tile([128, 512], F32, name="h1ps", tag="sbig")
                    for kc in range(4):
                        mm(pt, lhsT=w1(kc, 128 * i, 128 * (i + 1)),
                           rhs=attn_sb[kc][:, 512 * nh:512 * (nh + 1)],
                           start=(kc == 0), stop=(kc == 3))
                    tt = mt.tile([128, 512], BF16, name="mtt", tag="mtt")
                    nc.scalar.activation(tt, pt, Act.Exp, bias=b1_sb[:, i:i + 1])
                    t_t[(nh, i)] = tt
                    hf = mt.tile([128, 512], BF16, name="mtt", tag="mtt")
                    nc.vector.tensor_scalar_add(hf, pt, b1_sb[:, i:i + 1])
                    h1f[(nh, i)] = hf
            for nh in range(2):
                for i in range(4):
                    spt = mt.tile([128, 512], BF16, name="mtt", tag="mtt")
                    nc.scalar.activation(spt, t_t[(nh, i)], Act.Ln, bias=1.0)
                    sp_t[(nh, i)] = spt

            def mlp2(nh):
                for j in range(4):
                    pt = av.tile([128, 512], F32, name="h2ps", tag="av")
                    for kc in range(4):
                        mm(pt, lhsT=w2(kc, 128 * j, 128 * (j + 1)),
                           rhs=mish_sb[kc][:, 512 * nh:512 * (nh + 1)],
                           start=(kc == 0), stop=(kc == 3))
                    nc.vector.scalar_tensor_tensor(
                        out=out_sb[j][:, 512 * nh:512 * (nh + 1)],
                        in0=pt, scalar=b2_sb[:, j:j + 1],
                        in1=xbf(j, 512 * nh, 512 * (nh + 1)),
                        op0=Alu.add, op1=Alu.add)
                    nc.sync.dma_start(
                        out=out_d[128 * j:128 * (j + 1),
                                  512 * nh:512 * (nh + 1)],
                        in_=out_sb[j][:, 512 * nh:512 * (nh + 1)])

            # tanh shares a table set with exp, so per-nh chains cost no
            # extra loads; MLP2 of nh=0 overlaps the nh=1 chain on ACT.
            for nh in range(2):
                for i in range(4):
                    tht = mt.tile([128, 512], BF16, name="mtt", tag="mtt")
                    nc.scalar.activation(tht, sp_t[(nh, i)], Act.Tanh)
                    th_t[(nh, i)] = tht
                for i in range(4):
                    nc.vector.tensor_mul(
                        mish_sb[i][:, 512 * nh:512 * (nh + 1)],
                        h1f[(nh, i)], th_t[(nh, i)])
                mlp2(nh)

    nc.compile()
    return nc


def _get_nc():
    if "nc" not in _CACHE:
        _CACHE["nc"] = _build()
    return _CACHE["nc"]


def _interleave(a, cols):
    # [4*128, cols] -> [128, 4*cols] with (p, cols*k + c) = a[128*k + p, c]
    return np.ascontiguousarray(
        a.reshape(4, 128, cols).transpose(1, 0, 2).reshape(128, 4 * cols))


def _make_in_maps(inputs):
    x = np.asarray(inputs["x"], np.float32)
    PE = np.asarray(inputs["PE"], np.float32)
    Wq = np.asarray(inputs["Wq"], np.float32)
    bq = np.asarray(inputs["bq"], np.float32)
    Wk = np.asarray(inputs["Wk"], np.float32)
    bk = np.asarray(inputs["bk"], np.float32)
    Wv = np.asarray(inputs["Wv"], np.float32)
    bv = np.asarray(inputs["bv"], np.float32)
    W1 = np.asarray(inputs["W1"], np.float32)
    b1 = np.asarray(inputs["b1"], np.float32)
    W2 = np.asarray(inputs["W2"], np.float32)
    b2 = np.asarray(inputs["b2"], np.float32)

    import ml_dtypes
    BF = ml_dtypes.bfloat16
    s = np.float32(1.0 / np.sqrt(np.float32(32.0)))
    pef = PE.reshape(32, 1024)
    pe4 = np.tile(pef, (4, 1))  # [128, 1024], row = 32*j + d
    pe_tiles = [
        s * (pe4 + bq[0:128][:, None]),   # t=0 (q heads 0-3)
        s * (pe4 + bq[128:256][:, None]),  # t=1 (q heads 4-7)
        pe4 + bk[0:128][:, None],          # t=2 (k heads 0-3)
        pe4 + bk[128:256][:, None],        # t=3 (k heads 4-7)
    ]
    wqk_full = np.concatenate([s * Wq, Wk], axis=0).T  # [512 in_c, 512 out]
    wqk_kc = wqk_full.reshape(4, 128, 512)  # [kc, p, out]
    wqk_blocks = []
    for t in (0, 2, 1, 3):
        blk = wqk_kc[:, :, 128 * t:128 * (t + 1)]      # [kc, p, 128]
        wqk_blocks.append(blk.transpose(1, 0, 2).reshape(128, 512))
    wvt = _interleave(Wv.T, 512)
    w1t = _interleave(W1.T, 512)
    w2t = _interleave(W2.T, 512)

    # pack A: [wqk(t0) | wqk(t2) | pe(t0) | pe(t2)]  (gates the first exp)
    pka = np.ascontiguousarray(np.concatenate(
        [wqk_blocks[0], wqk_blocks[1], pe_tiles[0], pe_tiles[2]],
        axis=1).astype(BF))
    # pack B: [wv | wqk(t1) | wqk(t3) | pe(t1) | pe(t3) | w1 | w2]
    pkb = np.ascontiguousarray(np.concatenate(
        [wvt, wqk_blocks[2], wqk_blocks[3], pe_tiles[1], pe_tiles[3],
         w1t, w2t], axis=1).astype(BF))
    bvr = np.ascontiguousarray(bv.reshape(1, 512).astype(BF))
    b1c = np.ascontiguousarray(b1.astype(np.float32).reshape(4, 128).T)
    b2c = np.ascontiguousarray(b2.astype(np.float32).reshape(4, 128).T)

    xb = np.ascontiguousarray(x.reshape(8, 512, 1024))
    xbf = xb.astype(BF)
    shared = dict(pka=pka, pkb=pkb, bvr=bvr, b1=b1c, b2=b2c)
    return [dict(xbf=_interleave(xbf[i], 1024), **shared)
            for i in range(8)]


def _run(in_maps, trace=False, **kwargs):
    from concourse import bass_utils
    nc = _get_nc()
    return bass_utils.run_bass_kernel_spmd(
        nc, in_maps, core_ids=list(range(8)), trace=trace, **kwargs)


def kernel(**inputs):
    in_maps = _make_in_maps(inputs)
    res = _run(in_maps)
    out = np.stack([r["out"] for r in res.results], axis=0)
    return np.ascontiguousarray(out.reshape(8, 512, 32, 32).astype(np.float32))
